# revision 3
# baseline (speedup 1.0000x reference)
"""3-layer GCN (PyG GCNConv semantics) on 8 Trainium2 NeuronCores.

Strategy: destination-node sharding. Each core owns 6250 dst nodes.
Per layer: local z = h_local @ W (TensorE), AllGather z into a full
DRAM table, then per 128-dst tile: indirect-DMA gather of source rows,
norm-weighted one-hot matrix built on VectorE, TensorE matmul-accumulate
in PSUM (segment-sum as matmul). Bias+ReLU fused on ScalarE.
Edges are grouped by dst tile, sorted by src, padded to uniform chunk
count K so all 8 cores run one identical SPMD graph.
"""
import sys

sys.path.insert(0, "/opt/trn_rl_repo")
import numpy as np
from concourse import bass, bacc, mybir, tile
from concourse import bass_utils

NCORES = 8
N = 50000
FIN, HID, CLS = 128, 96, 32
VAL = N // NCORES            # 6250 valid dst rows per core
P = 128
TILES = (VAL + P - 1) // P   # 49
NL = TILES * P               # 6272 padded local rows
NT = NCORES * NL             # 50176 table rows
F32 = mybir.dt.float32
I32 = mybir.dt.int32

_cache: dict[int, "bacc.Bacc"] = {}


def _build(K: int):
    NCH = TILES * K
    nc = bacc.Bacc("TRN2", target_bir_lowering=False, debug=False,
                   num_devices=NCORES)
    xT_in = nc.dram_tensor("xT", [FIN, NL], F32, kind="ExternalInput")
    w1_in = nc.dram_tensor("w1", [FIN, HID], F32, kind="ExternalInput")
    w2_in = nc.dram_tensor("w2", [HID, HID], F32, kind="ExternalInput")
    w3_in = nc.dram_tensor("w3", [HID, CLS], F32, kind="ExternalInput")
    b1_in = nc.dram_tensor("b1c", [HID, 1], F32, kind="ExternalInput")
    b2_in = nc.dram_tensor("b2c", [HID, 1], F32, kind="ExternalInput")
    b3_in = nc.dram_tensor("b3r", [P, CLS], F32, kind="ExternalInput")
    iota_in = nc.dram_tensor("iota", [P, P], F32, kind="ExternalInput")
    srcs_in = nc.dram_tensor("srcs", [P, NCH], I32, kind="ExternalInput")
    dslot_in = nc.dram_tensor("dslot", [P, NCH], F32, kind="ExternalInput")
    enorm_in = nc.dram_tensor("enorm", [P, NCH], F32, kind="ExternalInput")
    out_d = nc.dram_tensor("out", [NL, CLS], F32, kind="ExternalOutput")

    with tile.TileContext(nc) as tc:
        with tc.tile_pool(name="const", bufs=1) as cp, \
             tc.tile_pool(name="work", bufs=4) as wp, \
             tc.tile_pool(name="gath", bufs=8) as gp, \
             tc.tile_pool(name="psum", bufs=2, space="PSUM") as pp, \
             tc.tile_pool(name="dram", bufs=1, space="DRAM") as dp:
            xT = cp.tile([FIN, NL], F32)
            w1 = cp.tile([FIN, HID], F32)
            w2 = cp.tile([HID, HID], F32)
            w3 = cp.tile([HID, CLS], F32)
            b1c = cp.tile([HID, 1], F32)
            b2c = cp.tile([HID, 1], F32)
            b3r = cp.tile([P, CLS], F32)
            iota = cp.tile([P, P], F32)
            srcs = cp.tile([P, NCH], I32)
            dslot = cp.tile([P, NCH], F32)
            enorm = cp.tile([P, NCH], F32)
            h1T = cp.tile([HID, NL], F32)
            h2T = cp.tile([HID, NL], F32)
            for sb_t, dr in ((xT, xT_in), (w1, w1_in), (w2, w2_in),
                             (w3, w3_in), (b1c, b1_in), (b2c, b2_in),
                             (b3r, b3_in), (iota, iota_in), (srcs, srcs_in),
                             (dslot, dslot_in), (enorm, enorm_in)):
                nc.sync.dma_start(sb_t[:], dr[:])

            ag1 = dp.tile([NL, HID], F32)
            T1 = dp.tile([NT, HID], F32, addr_space="Shared")
            ag2 = dp.tile([NL, HID], F32)
            T2 = dp.tile([NT, HID], F32, addr_space="Shared")
            ag3 = dp.tile([NL, CLS], F32)
            T3 = dp.tile([NT, CLS], F32, addr_space="Shared")

            def z_layer(hT_ap, w_ap, fout, ag_ap):
                # z_tile [128 nodes, fout] = hT_cols.T @ W ; DMA to allgather in
                for t in range(TILES):
                    zp = pp.tile([P, fout], F32, space="PSUM", name="zp")
                    nc.tensor.matmul(out=zp[:], lhsT=hT_ap[:, t * P:(t + 1) * P],
                                     rhs=w_ap[:], start=True, stop=True)
                    zs = wp.tile([P, fout], F32, name="zs")
                    nc.scalar.activation(out=zs[:], in_=zp[:],
                                         func=mybir.ActivationFunctionType.Copy)
                    nc.sync.dma_start(ag_ap[t * P:(t + 1) * P, :], zs[:])

            def allgather(ag_ap, t_ap):
                nc.gpsimd.collective_compute(
                    "AllGather", mybir.AluOpType.bypass,
                    replica_groups=[list(range(NCORES))],
                    ins=[ag_ap.opt()], outs=[t_ap.opt()],
                )

            def agg_layer_T(t_ap, fout, hT_out, bc_ap):
                # out hT_out[:, tile] [fout, 128] = relu(msgs.T @ S + b)
                for t in range(TILES):
                    ap_ps = pp.tile([fout, P], F32, space="PSUM", name="aggp")
                    for k in range(K):
                        col = t * K + k
                        g = gp.tile([P, fout], F32, name="g")
                        nc.gpsimd.indirect_dma_start(
                            out=g[:], out_offset=None, in_=t_ap[:],
                            in_offset=bass.IndirectOffsetOnAxis(
                                ap=srcs[:, col:col + 1], axis=0))
                        S = gp.tile([P, P], F32, name="S")
                        nc.vector.tensor_scalar(
                            out=S[:], in0=iota[:],
                            scalar1=dslot[:, col:col + 1],
                            scalar2=enorm[:, col:col + 1],
                            op0=mybir.AluOpType.is_equal,
                            op1=mybir.AluOpType.mult)
                        nc.tensor.matmul(out=ap_ps[:], lhsT=g[:], rhs=S[:],
                                         start=(k == 0), stop=(k == K - 1))
                    nc.scalar.activation(out=hT_out[:, t * P:(t + 1) * P],
                                         in_=ap_ps[:],
                                         func=mybir.ActivationFunctionType.Relu,
                                         bias=bc_ap[:, :1])

            z_layer(xT, w1, HID, ag1)
            allgather(ag1, T1)
            agg_layer_T(T1, HID, h1T, b1c)

            z_layer(h1T, w2, HID, ag2)
            allgather(ag2, T2)
            agg_layer_T(T2, HID, h2T, b2c)

            z_layer(h2T, w3, CLS, ag3)
            allgather(ag3, T3)
            # layer 3: row-major aggregation + bias + log_softmax
            for t in range(TILES):
                ap3 = pp.tile([P, CLS], F32, space="PSUM", name="ap3")
                for k in range(K):
                    col = t * K + k
                    g3 = gp.tile([P, CLS], F32, name="g3")
                    nc.gpsimd.indirect_dma_start(
                        out=g3[:], out_offset=None, in_=T3[:],
                        in_offset=bass.IndirectOffsetOnAxis(
                            ap=srcs[:, col:col + 1], axis=0))
                    S3 = gp.tile([P, P], F32, name="S3")
                    nc.vector.tensor_scalar(
                        out=S3[:], in0=iota[:],
                        scalar1=dslot[:, col:col + 1],
                        scalar2=enorm[:, col:col + 1],
                        op0=mybir.AluOpType.is_equal,
                        op1=mybir.AluOpType.mult)
                    nc.tensor.matmul(out=ap3[:], lhsT=S3[:], rhs=g3[:],
                                     start=(k == 0), stop=(k == K - 1))
                z3 = wp.tile([P, CLS], F32, name="z3")
                nc.vector.tensor_tensor(out=z3[:], in0=ap3[:], in1=b3r[:],
                                        op=mybir.AluOpType.add)
                m3 = wp.tile([P, 1], F32, name="m3")
                nc.vector.tensor_reduce(out=m3[:], in_=z3[:],
                                        axis=mybir.AxisListType.X,
                                        op=mybir.AluOpType.max)
                nm3 = wp.tile([P, 1], F32, name="nm3")
                nc.vector.tensor_scalar(out=nm3[:], in0=m3[:], scalar1=-1.0,
                                        scalar2=None,
                                        op0=mybir.AluOpType.mult)
                e3 = wp.tile([P, CLS], F32, name="e3")
                s3 = wp.tile([P, 1], F32, name="s3")
                nc.scalar.activation(out=e3[:], in_=z3[:],
                                     func=mybir.ActivationFunctionType.Exp,
                                     bias=nm3[:, :1], accum_out=s3[:, :1])
                ls3 = wp.tile([P, 1], F32, name="ls3")
                nc.scalar.activation(out=ls3[:], in_=s3[:],
                                     func=mybir.ActivationFunctionType.Ln)
                o3 = wp.tile([P, CLS], F32, name="o3")
                nc.vector.tensor_scalar(out=o3[:], in0=z3[:],
                                        scalar1=nm3[:, :1], scalar2=ls3[:, :1],
                                        op0=mybir.AluOpType.add,
                                        op1=mybir.AluOpType.subtract)
                nc.sync.dma_start(out_d[t * P:(t + 1) * P, :], o3[:])
    nc.compile()
    return nc


def _prep(x, edge_index, W1, b1, W2, b2, W3, b3):
    src = np.concatenate([edge_index[0], np.arange(N, dtype=np.int64)])
    dst = np.concatenate([edge_index[1], np.arange(N, dtype=np.int64)])
    deg = np.bincount(dst, minlength=N).astype(np.float64)
    dinv = 1.0 / np.sqrt(deg)
    norm = (dinv[src] * dinv[dst]).astype(np.float32)

    core = dst // VAL
    dloc = dst - core * VAL
    tile_id = dloc // P
    slot = (dloc % P).astype(np.float32)
    gtile = (core * TILES + tile_id).astype(np.int64)
    src_tbl = ((src // VAL) * NL + (src % VAL)).astype(np.int32)

    order = np.lexsort((src_tbl, gtile))
    counts = np.bincount(gtile, minlength=NCORES * TILES)
    K = int(np.ceil(counts.max() / P))
    tot = NCORES * TILES
    srcs_p = np.zeros((tot, K * P), np.int32)
    slot_p = np.zeros((tot, K * P), np.float32)
    norm_p = np.zeros((tot, K * P), np.float32)
    starts = np.zeros(tot + 1, np.int64)
    np.cumsum(counts, out=starts[1:])
    rows = gtile[order]
    cols = np.arange(len(src)) - starts[rows]
    srcs_p[rows, cols] = src_tbl[order]
    slot_p[rows, cols] = slot[order]
    norm_p[rows, cols] = norm[order]

    iota = np.tile(np.arange(P, dtype=np.float32), (P, 1))
    in_maps = []
    for c in range(NCORES):
        xT = np.zeros((FIN, NL), np.float32)
        xT[:, :VAL] = x[c * VAL:(c + 1) * VAL].T
        blk = slice(c * TILES, (c + 1) * TILES)
        in_maps.append({
            "xT": xT,
            "w1": np.ascontiguousarray(W1, np.float32),
            "w2": np.ascontiguousarray(W2, np.float32),
            "w3": np.ascontiguousarray(W3, np.float32),
            "b1c": np.ascontiguousarray(b1[:, None], np.float32),
            "b2c": np.ascontiguousarray(b2[:, None], np.float32),
            "b3r": np.tile(np.asarray(b3, np.float32), (P, 1)),
            "iota": iota,
            "srcs": np.ascontiguousarray(srcs_p[blk].reshape(TILES * K, P).T),
            "dslot": np.ascontiguousarray(slot_p[blk].reshape(TILES * K, P).T),
            "enorm": np.ascontiguousarray(norm_p[blk].reshape(TILES * K, P).T),
        })
    return K, in_maps


def kernel(x, edge_index, W1, b1, W2, b2, W3, b3):
    x = np.asarray(x, np.float32)
    edge_index = np.asarray(edge_index, np.int64)
    K, in_maps = _prep(x, edge_index, np.asarray(W1), np.asarray(b1),
                       np.asarray(W2), np.asarray(b2), np.asarray(W3),
                       np.asarray(b3))
    if K not in _cache:
        _cache[K] = _build(K)
    res = bass_utils.run_bass_kernel_spmd(_cache[K], in_maps,
                                          core_ids=list(range(NCORES)))
    out = np.concatenate([res.results[c]["out"][:VAL] for c in range(NCORES)],
                         axis=0)
    return np.ascontiguousarray(out, np.float32)
